# revision 2
# baseline (speedup 1.0000x reference)
"""Trainium2 Bass kernel for DecGridDeepVPN (gnn_message_passing), 8-core SPMD.

Math (per batch row b, agents n=0..19):
  nsc[b]  = action_count[b].reshape(405) @ INFLOW            # [81]
  ir[b]   = sum_d min(nsc[b,d], demand[b,d])                 # scalar
  feat    = [nsc | onehot81(loc[b,n]) | la[b,n] | onehot20(n)]   # 187
  h0 = relu(feat @ W0); h1 = relu(h0 @ W1); out[b,n] = h1@W2 + b2 + ir[b]

Sharding: pure data-parallel over batch (4096 rows/core). Inside each core
everything runs feature-major (features on partitions, batch streaming on the
free axis). Columns of the MLP stage are (b, g) pairs with g = n//4; the four
agent classes j = n%4 occupy four 32-row partition blocks, so layers 0/1/2
run as single block-diagonal matmuls with K=M=128 (the PE touches each column
once per layer instead of four times).

Layer-0 decomposition (W0 = [W0a | W0b | W0c | W0d] over the feature concat):
  zbase = W0a.T @ nscT        broadcast over agents via a 0-step rhs AP
  zg    = W0b[loc]            embedding-table lookup; this toolchain exposes
                              no usable device gather (gpsimd custom ISA ops
                              don't compile here), so the lookup is done as
                              host-side layout prep and streamed in bf16
  la/agent one-hot terms      one K=25 block-diagonal matmul
ir (+b2) is reduced on device (ones-vector matmul) and added exactly (fp32)
on the host during unsharding.

All streamed operands are bf16 (inputs are O(1) uniforms; worst-case output
error ~1e-3 relative); PSUM accumulation and the outputs are fp32.
"""

import dataclasses as _dc

import numpy as np
import ml_dtypes

import concourse.bass as bass
import concourse.mybir as mybir
from concourse.bass_utils import run_bass_kernel_spmd

BF16 = ml_dtypes.bfloat16
F32 = mybir.dt.float32
BF = mybir.dt.bfloat16

S = 81          # grid states
GRID = 9
N_AG = 20       # agents
A = 5           # actions
B = 32768
NCORES = 8
BC = B // NCORES      # 4096 batch rows per core
NBB = 8               # b-blocks per core (phase N)
BB = BC // NBB        # 512
NG = BC * 5           # 20480 (b, g) columns per core
CB = 510              # col-block: multiple of 5, fits one PSUM bank
NCB = (NG + CB - 1) // CB   # 41 (last block = 80 cols)
NGP = NCB * CB        # padded column count


def _grid_inflow():
    moves = [(0, 0), (-1, 0), (1, 0), (0, -1), (0, 1)]
    inflow = np.zeros((S * A, S), np.float32)
    for s in range(S):
        r, c = divmod(s, GRID)
        for a, (dr, dc) in enumerate(moves):
            nr, nc_ = r + dr, c + dc
            d = nr * GRID + nc_ if (0 <= nr < GRID and 0 <= nc_ < GRID) else s
            inflow[s * A + a, d] = 1.0
    return inflow


def _cw(k):
    return min(CB, NG - k * CB)


def _build(reps=1):
    nc = bass.Bass()

    acT = nc.declare_dram_parameter("acT", [4, NBB, 128, BB], BF, isOutput=False)
    demT = nc.declare_dram_parameter("demT", [NBB, S, BB], BF, isOutput=False)
    zgb = nc.declare_dram_parameter("zgb", [NCB, 128, CB], BF, isOutput=False)
    rhs0 = nc.declare_dram_parameter("rhs0", [NCB, 25, CB], BF, isOutput=False)
    wIN = nc.declare_dram_parameter("wIN", [4, 128, S], BF, isOutput=False)
    wA = nc.declare_dram_parameter("wA", [S, 32], BF, isOutput=False)
    w0le = nc.declare_dram_parameter("w0le", [25, 128], BF, isOutput=False)
    wREP = nc.declare_dram_parameter("wREP", [32, 128], BF, isOutput=False)
    w1 = nc.declare_dram_parameter("w1", [128, 128], BF, isOutput=False)
    w2i = nc.declare_dram_parameter("w2i", [128, 4], BF, isOutput=False)
    ones81 = nc.declare_dram_parameter("ones81", [S, 1], BF, isOutput=False)
    out4 = nc.declare_dram_parameter("out4", [NCB, 4, CB], F32, isOutput=True)
    irO = nc.declare_dram_parameter("irO", [1, BC], F32, isOutput=True)

    NW = 10  # weight DMAs

    from contextlib import ExitStack
    ctx = ExitStack()
    with ctx:
        s_ac0 = ctx.enter_context(nc.sbuf_tensor([128, 4 * BB], BF))
        s_ac1 = ctx.enter_context(nc.sbuf_tensor([128, 4 * BB], BF))
        s_ac2 = ctx.enter_context(nc.sbuf_tensor([128, 4 * BB], BF))
        s_ac3 = ctx.enter_context(nc.sbuf_tensor([128, 4 * BB], BF))
        s_dem = ctx.enter_context(nc.sbuf_tensor([S, BC], BF))
        s_nscT = ctx.enter_context(nc.sbuf_tensor([S, BC], BF))
        s_srv = ctx.enter_context(nc.sbuf_tensor([S, BC], BF))
        s_zb = ctx.enter_context(nc.sbuf_tensor([32, BC], BF))
        s_ir = ctx.enter_context(nc.sbuf_tensor([1, BC], F32))
        s_zg = ctx.enter_context(nc.sbuf_tensor([128, 2 * CB], BF))
        s_r0 = ctx.enter_context(nc.sbuf_tensor([25, 2 * CB], BF))
        s_h0p = ctx.enter_context(nc.sbuf_tensor([128, 2 * CB], BF))
        s_h0 = ctx.enter_context(nc.sbuf_tensor([128, 2 * CB], BF))
        s_h1 = ctx.enter_context(nc.sbuf_tensor([128, 2 * CB], BF))
        s_o4 = ctx.enter_context(nc.sbuf_tensor([4, 4 * CB], F32))
        s_wIN0 = ctx.enter_context(nc.sbuf_tensor([128, S], BF))
        s_wIN1 = ctx.enter_context(nc.sbuf_tensor([128, S], BF))
        s_wIN2 = ctx.enter_context(nc.sbuf_tensor([128, S], BF))
        s_wIN3 = ctx.enter_context(nc.sbuf_tensor([128, S], BF))
        s_wA = ctx.enter_context(nc.sbuf_tensor([S, 32], BF))
        s_w0le = ctx.enter_context(nc.sbuf_tensor([25, 128], BF))
        s_wREP = ctx.enter_context(nc.sbuf_tensor([32, 128], BF))
        s_w1 = ctx.enter_context(nc.sbuf_tensor([128, 128], BF))
        s_w2i = ctx.enter_context(nc.sbuf_tensor([128, 4], BF))
        s_ones = ctx.enter_context(nc.sbuf_tensor([S, 1], BF))
        psum = ctx.enter_context(nc.psum_tensor([128, 4096], F32))
        d_w = ctx.enter_context(nc.semaphore())
        d_ac = [ctx.enter_context(nc.semaphore(f'd_ac{_i}')) for _i in range(4)]
        d_dem = ctx.enter_context(nc.semaphore())
        d_m = [ctx.enter_context(nc.semaphore(f'd_m{_i}')) for _i in range(2)]
        d_out = [ctx.enter_context(nc.semaphore(f'd_out{_i}')) for _i in range(4)]
        d_ir = ctx.enter_context(nc.semaphore())
        t_nsc = ctx.enter_context(nc.semaphore())
        t_zbir = ctx.enter_context(nc.semaphore())
        t_h0 = ctx.enter_context(nc.semaphore())
        t_h1 = ctx.enter_context(nc.semaphore())
        t_o = ctx.enter_context(nc.semaphore())
        v_srv = ctx.enter_context(nc.semaphore())
        v_h0 = ctx.enter_context(nc.semaphore())
        sc_nsc = ctx.enter_context(nc.semaphore())
        sc_zbir = ctx.enter_context(nc.semaphore())
        sc_h0 = ctx.enter_context(nc.semaphore())
        sc_h1 = ctx.enter_context(nc.semaphore())
        sc_o = ctx.enter_context(nc.semaphore())
        block = ctx.enter_context(nc.Block())
        s_acs = [s_ac0, s_ac1, s_ac2, s_ac3]
        s_wINs = [s_wIN0, s_wIN1, s_wIN2, s_wIN3]

        def p_nsc(i):
            return psum[0:S, (i % 2) * 512:(i % 2) * 512 + BB]

        def p_misc(i):
            return psum[:, 1024 + (i % 2) * 512: 1024 + (i % 2) * 512 + BB]

        def p_h0(gk):
            return psum[:, (gk % 2) * 512:(gk % 2) * 512 + _cw(gk % NCB)]

        def p_h1(gk):
            return psum[:, 1024 + (gk % 2) * 512:1024 + (gk % 2) * 512 + _cw(gk % NCB)]

        def p_o(gk):
            return psum[0:4, 2048 + (gk % 4) * 512:2048 + (gk % 4) * 512 + _cw(gk % NCB)]

        @block.sync
        def _(sync):
            for i in range(4):
                sync.dma_start(s_wINs[i][:, :], wIN[i]).then_inc(d_w, 16)
            sync.dma_start(s_wA[:, :], wA[:, :]).then_inc(d_w, 16)
            sync.dma_start(s_w0le[:, :], w0le[:, :]).then_inc(d_w, 16)
            sync.dma_start(s_wREP[:, :], wREP[:, :]).then_inc(d_w, 16)
            sync.dma_start(s_w1[:, :], w1[:, :]).then_inc(d_w, 16)
            sync.dma_start(s_w2i[:, :], w2i[:, :]).then_inc(d_w, 16)
            sync.dma_start(s_ones[:, :], ones81[:, :]).then_inc(d_w, 16)
            for r in range(reps):
                if r >= 1:
                    sync.wait_ge(v_srv, NBB * r)  # s_dem consumed by prev rep
                for i in range(NBB):
                    sync.dma_start(
                        s_dem[:, i * BB:(i + 1) * BB], demT[i]
                    ).then_inc(d_dem, 16)
            for gi in range(reps * NBB):
                i = gi % NBB
                if gi >= 4:
                    sync.wait_ge(t_nsc, gi - 3)  # ac buffer slot free
                sl = slice((gi % 4) * BB, (gi % 4) * BB + BB)
                for c in range(4):
                    sync.dma_start(s_acs[c][:, sl], acT[c, i]).then_inc(
                        d_ac[gi % 4], 16)
            for gk in range(reps * NCB):
                k = gk % NCB
                cw = _cw(k)
                sl = slice((gk % 2) * CB, (gk % 2) * CB + cw)
                if gk >= 2:
                    sync.wait_ge(v_h0, 2 * (gk - 1))  # zg/r0 slot consumed
                sync.dma_start(s_zg[:, sl], zgb[k, :, 0:cw]).then_inc(
                    d_m[gk % 2], 16)
                sync.dma_start(s_r0[:, sl], rhs0[k, :, 0:cw]).then_inc(
                    d_m[gk % 2], 16)

        @block.gpsimd
        def _(gpsimd):
            for gk in range(reps * NCB):
                k = gk % NCB
                gpsimd.wait_ge(sc_o, gk + 1)
                cw = _cw(k)
                sl = slice((gk % 4) * CB, (gk % 4) * CB + cw)
                gpsimd.dma_start(out4[k, :, 0:cw], s_o4[:, sl]).then_inc(
                    d_out[gk % 4], 16)
                if k == NCB - 1:
                    r = gk // NCB
                    gpsimd.wait_ge(sc_zbir, 2 * NBB * (r + 1))
                    gpsimd.dma_start(irO[:, :], s_ir[:, :]).then_inc(d_ir, 16)

        @block.tensor
        def _(tensor):
            tensor.wait_ge(d_w, NW * 16)
            for r in range(reps):
                base_i = r * NBB
                base_k = r * NCB
                # ---- phase N: nscT ----
                for i in range(NBB):
                    gi = base_i + i
                    tensor.wait_ge(d_ac[gi % 4], (gi // 4 + 1) * 64)
                    if r >= 1 and i < 2:
                        tensor.wait_ge(sc_h0, NCB * r)  # banks 0-1 freed of h0
                    if gi >= 2:
                        tensor.wait_ge(sc_nsc, gi - 1)  # psum buf free
                    sl = slice((gi % 4) * BB, (gi % 4) * BB + BB)
                    for c in range(4):
                        mm = nc.tensor.matmul(
                            p_nsc(gi), s_wINs[c][:, :], s_acs[c][:, sl],
                            start=(c == 0), stop=(c == 3),
                        )
                    mm.then_inc(t_nsc, 1)
                # ---- phase N: ir reduce + zbase ----
                for i in range(NBB):
                    gi = base_i + i
                    bsl = slice(i * BB, i * BB + BB)
                    pm = p_misc(gi)
                    tensor.wait_ge(v_srv, gi + 1)
                    if r >= 1 and i < 2:
                        tensor.wait_ge(sc_h1, NCB * r)  # banks 2-3 freed of h1
                    if gi >= 2:
                        tensor.wait_ge(sc_zbir, 2 * (gi - 1))  # psum buf free
                    nc.tensor.matmul(
                        pm[0:1, :], s_ones[:, :], s_srv[:, bsl],
                        start=True, stop=True,
                    )
                    tensor.wait_ge(sc_nsc, gi + 1)
                    nc.tensor.matmul(
                        pm[32:64, :], s_wA[:, :], s_nscT[:, bsl],
                        start=True, stop=True,
                    ).then_inc(t_zbir, 1)
                # ---- phase M, software-pipelined ----
                tensor.wait_ge(sc_zbir, 2 * NBB * (r + 1))
                tensor.wait_ge(sc_nsc, NBB * (r + 1))  # banks 0-1 freed of nsc
                for kk in range(NCB + 2):
                    if kk < NCB:
                        k = kk
                        gk = base_k + k
                        cw = _cw(k)
                        sl = slice((gk % 2) * CB, (gk % 2) * CB + cw)
                        tensor.wait_ge(d_m[gk % 2], (gk // 2 + 1) * 32)
                        if gk >= 2:
                            tensor.wait_ge(sc_h0, gk - 1)  # p_h0 buf free
                        nc.tensor.matmul(
                            p_h0(gk), s_w0le[:, :], s_r0[:, sl],
                            start=True, stop=False,
                        )
                        b0 = (k * CB) // 5
                        nb = cw // 5
                        zb_ap = s_zb[:, b0:b0 + nb]
                        zb_ap = _dc.replace(
                            zb_ap, ap=[zb_ap.ap[0], [1, nb], [0, 5]])
                        nc.tensor.matmul(
                            p_h0(gk), s_wREP[:, :], zb_ap,
                            start=False, stop=True,
                        ).then_inc(t_h0, 1)
                    if 1 <= kk and kk - 1 < NCB:
                        k = kk - 1
                        gk = base_k + k
                        cw = _cw(k)
                        sl = slice((gk % 2) * CB, (gk % 2) * CB + cw)
                        tensor.wait_ge(v_h0, 2 * gk + 2)  # h0 ready
                        if gk >= 2:
                            tensor.wait_ge(sc_h1, gk - 1)  # p_h1 buf free
                        nc.tensor.matmul(
                            p_h1(gk), s_w1[:, :], s_h0[:, sl],
                            start=True, stop=True,
                        ).then_inc(t_h1, 1)
                    if 2 <= kk and kk - 2 < NCB:
                        k = kk - 2
                        gk = base_k + k
                        cw = _cw(k)
                        sl = slice((gk % 2) * CB, (gk % 2) * CB + cw)
                        tensor.wait_ge(sc_h1, gk + 1)
                        if gk >= 4:
                            tensor.wait_ge(d_out[gk % 4], ((gk - 4) // 4 + 1) * 16)
                        nc.tensor.matmul(
                            p_o(gk), s_w2i[:, :], s_h1[:, sl],
                            start=True, stop=True,
                        ).then_inc(t_o, 1)

        @block.vector
        def _(vector):
            for gi in range(reps * NBB):
                i = gi % NBB
                r = gi // NBB
                bsl = slice(i * BB, i * BB + BB)
                if i == 0:
                    vector.wait_ge(d_dem, NBB * 16 * (r + 1))
                vector.wait_ge(sc_nsc, gi + 1)
                if gi >= NBB:
                    vector.wait_ge(t_zbir, gi - NBB + 1)  # s_srv consumed
                nc.vector.tensor_tensor(
                    s_srv[:, bsl], s_nscT[:, bsl], s_dem[:, bsl],
                    mybir.AluOpType.min,
                ).then_inc(v_srv, 1)
                if i == NBB - 1:
                    # phase M of this rep
                    for k in range(NCB):
                        gk = r * NCB + k
                        cw = _cw(k)
                        sl = slice((gk % 2) * CB, (gk % 2) * CB + cw)
                        vector.wait_ge(sc_h0, gk + 1)
                        vector.wait_ge(d_m[gk % 2], (gk // 2 + 1) * 32)
                        if gk >= 2:
                            vector.wait_ge(t_h1, gk - 1)  # s_h0 slot consumed
                        nc.vector.tensor_tensor(
                            s_h0[:, sl], s_h0p[:, sl], s_zg[:, sl],
                            mybir.AluOpType.add,
                        ).then_inc(v_h0, 1)
                        vector.wait_ge(v_h0, 2 * gk + 1)  # same-engine RAW
                        nc.vector.tensor_scalar_max(
                            s_h0[:, sl], s_h0[:, sl], 0.0
                        ).then_inc(v_h0, 1)

        @block.scalar
        def _(scalar):
            AF = mybir.ActivationFunctionType
            for r in range(reps):
                for i in range(NBB):
                    gi = r * NBB + i
                    bsl = slice(i * BB, i * BB + BB)
                    scalar.wait_ge(t_nsc, gi + 1)
                    if gi >= NBB:
                        # s_nscT consumed by zb-mm + vector min of prev rep
                        scalar.wait_ge(t_zbir, gi - NBB + 1)
                        scalar.wait_ge(v_srv, gi - NBB + 1)
                    nc.scalar.copy(s_nscT[:, bsl], p_nsc(gi)).then_inc(sc_nsc, 1)
                for i in range(NBB):
                    gi = r * NBB + i
                    bsl = slice(i * BB, i * BB + BB)
                    scalar.wait_ge(t_zbir, gi + 1)
                    if r >= 1 and i == 0:
                        scalar.wait_ge(t_h0, NCB * r)   # s_zb consumed
                        scalar.wait_ge(d_ir, 16 * r)    # s_ir dma'd out
                    pm = p_misc(gi)
                    nc.scalar.copy(s_ir[:, bsl], pm[0:1, :]).then_inc(sc_zbir, 1)
                    nc.scalar.copy(s_zb[:, bsl], pm[32:64, :]).then_inc(sc_zbir, 1)
                # skewed phase M: h0p(kk) | relu-h1(kk-1) | out(kk-2)
                for kk in range(NCB + 2):
                    if kk < NCB:
                        gk = r * NCB + kk
                        cw = _cw(kk)
                        sl = slice((gk % 2) * CB, (gk % 2) * CB + cw)
                        scalar.wait_ge(t_h0, gk + 1)
                        if gk >= 2:
                            scalar.wait_ge(v_h0, 2 * gk - 3)  # h0p slot free
                        nc.scalar.copy(s_h0p[:, sl], p_h0(gk)).then_inc(sc_h0, 1)
                    if 1 <= kk and kk - 1 < NCB:
                        k = kk - 1
                        gk = r * NCB + k
                        cw = _cw(k)
                        sl = slice((gk % 2) * CB, (gk % 2) * CB + cw)
                        scalar.wait_ge(t_h1, gk + 1)
                        nc.scalar.activation(
                            s_h1[:, sl], p_h1(gk), AF.Relu
                        ).then_inc(sc_h1, 1)
                    if 2 <= kk and kk - 2 < NCB:
                        k = kk - 2
                        gk = r * NCB + k
                        cw = _cw(k)
                        scalar.wait_ge(t_o, gk + 1)
                        if gk >= 4:
                            scalar.wait_ge(d_out[gk % 4], ((gk - 4) // 4 + 1) * 16)
                        osl = slice((gk % 4) * CB, (gk % 4) * CB + cw)
                        nc.scalar.copy(s_o4[:, osl], p_o(gk)).then_inc(sc_o, 1)

    return nc


_NC = {}


def _get_nc(reps=1):
    if reps not in _NC:
        _NC[reps] = _build(reps)
    return _NC[reps]


def _prep_core(obs, ac, la, W0, zg_tab):
    """Host-side layout prep for one core's batch slice (all numpy)."""
    bc = obs.shape[0]
    out = {}
    # acT: [405, bc] padded to [512, bc] -> [4, NBB, 128, BB]
    acT = np.zeros((512, bc), np.float32)
    acT[:405] = ac.reshape(bc, 405).T
    out["acT"] = np.ascontiguousarray(
        acT.reshape(4, 128, NBB, BB).transpose(0, 2, 1, 3)
    ).astype(BF16)
    # demT: [NBB, 81, BB]
    dem = obs[:, S:2 * S].T  # [81, bc]
    out["demT"] = np.ascontiguousarray(
        dem.reshape(S, NBB, BB).transpose(1, 0, 2)
    ).astype(BF16)
    # zg: [128, NG] stacked by agent class, then col-blocked [NCB, 128, CB]
    loc = obs[:, 2 * S:2 * S + N_AG].astype(np.int64)  # [bc, 20]
    zst = np.empty((128, bc * 5), np.float32)
    for j in range(4):
        lj = loc[:, j::4].reshape(-1)            # cols (b, g)
        zst[32 * j:32 * j + 32] = zg_tab[lj].T   # [32, bc*5]
    zstp = np.zeros((128, NGP), np.float32)
    zstp[:, :bc * 5] = zst
    out["zgb"] = np.ascontiguousarray(
        zstp.reshape(128, NCB, CB).transpose(1, 0, 2)
    ).astype(BF16)
    # rhs0: rows 0-19 la packed, rows 20-24 g-onehot; [NCB, 25, CB]
    r0 = np.zeros((25, NGP), np.float32)
    for j in range(4):
        r0[5 * j:5 * j + 5, :bc * 5] = (
            la[:, j::4, :].transpose(2, 0, 1).reshape(5, bc * 5)
        )
    r0[20:25, :bc * 5] = np.tile(np.eye(5, dtype=np.float32), (1, bc))
    out["rhs0"] = np.ascontiguousarray(
        r0.reshape(25, NCB, CB).transpose(1, 0, 2)
    ).astype(BF16)
    return out


def kernel(obs, action_count, local_actions, W0, W1, W2, b2):
    obs = np.asarray(obs, np.float32)
    action_count = np.asarray(action_count, np.float32)
    local_actions = np.asarray(local_actions, np.float32)
    W0 = np.asarray(W0, np.float32)
    W1 = np.asarray(W1, np.float32)
    W2 = np.asarray(W2, np.float32)
    b2 = np.asarray(b2, np.float32)

    # ---- weight repacking (shared across cores) ----
    W0a, W0b = W0[0:S], W0[S:2 * S]           # [81,32], [81,32]
    W0c, W0d = W0[2 * S:2 * S + A], W0[2 * S + A:]  # [5,32], [20,32]
    inflow = _grid_inflow()
    wIN = np.zeros((512, S), np.float32)
    wIN[:405] = inflow
    wIN = wIN.reshape(4, 128, S).astype(BF16)
    w0le = np.zeros((25, 128), np.float32)
    for j in range(4):
        w0le[5 * j:5 * j + 5, 32 * j:32 * j + 32] = W0c
        for e in range(5):
            w0le[20 + e, 32 * j:32 * j + 32] = W0d[4 * e + j]
    wREP = np.tile(np.eye(32, dtype=np.float32), (1, 4))
    w1b = np.zeros((128, 128), np.float32)
    for j in range(4):
        w1b[32 * j:32 * j + 32, 32 * j:32 * j + 32] = W1
    w2i = np.zeros((128, 4), np.float32)
    for j in range(4):
        w2i[32 * j:32 * j + 32, j] = W2[:, 0]
    consts = {
        "wIN": wIN,
        "wA": W0a.astype(BF16),
        "w0le": w0le.astype(BF16),
        "wREP": wREP.astype(BF16),
        "w1": w1b.astype(BF16),
        "w2i": w2i.astype(BF16),
        "ones81": np.ones((S, 1), np.float32).astype(BF16),
    }

    in_maps = []
    for c in range(NCORES):
        bsl = slice(c * BC, (c + 1) * BC)
        m = _prep_core(
            obs[bsl], action_count[bsl], local_actions[bsl], W0, W0b
        )
        m.update(consts)
        in_maps.append(m)

    nc = _get_nc()
    res = run_bass_kernel_spmd(nc, in_maps, list(range(NCORES)))
    global LAST_RESULTS
    LAST_RESULTS = res

    out = np.empty((B, N_AG), np.float32)
    for c in range(NCORES):
        r = res.results[c]
        o4 = r["out4"].transpose(1, 0, 2).reshape(4, NGP)[:, :NG]  # [4, (b,g)]
        o4 = o4.reshape(4, BC, 5)
        ob = o4.transpose(1, 2, 0).reshape(BC, N_AG)  # out[b, 4g+j]
        ob += r["irO"][0][:, None] + b2[0]
        out[c * BC:(c + 1) * BC] = ob
    return out



# revision 13
# speedup vs baseline: 1.8434x; 1.8434x over previous
"""Trainium2 Bass kernel for DecGridDeepVPN (gnn_message_passing), 8-core SPMD.

Math (per batch row b, agents n=0..19):
  nsc[b]  = action_count[b].reshape(405) @ INFLOW            # [81]
  ir[b]   = sum_d min(nsc[b,d], demand[b,d])                 # scalar
  feat    = [nsc | onehot81(loc[b,n]) | la[b,n] | onehot20(n)]   # 187
  h0 = relu(feat @ W0); h1 = relu(h0 @ W1); out[b,n] = h1@W2 + b2 + ir[b]

Sharding: pure data-parallel over batch (4096 rows/core). Feature-major on
device: features on partitions, batch streaming on the free axis. MLP columns
are (b, g) pairs with g = n//4; agent classes j = n%4 occupy four 32-row
partition blocks, so all layers are K=128 block-diagonal matmuls.

v2 design (from trace analysis of v1: PE 80% cold-throttled, scalar engine
at 98us of PSUM-copy work, 174 small DMAs):
  - every matmul is K-padded to 128 with zeroed SBUF rows / weight rows so
    the PE never switches tile mode (no drain, stays HAM-warm)
  - the h0 = relu(presum + zg) add reads PSUM directly on the vector engine
    (no staging copy); relu-h1 on scalar; outputs of 4 consecutive column
    blocks are accumulated into one PSUM bank via sparse-M W2 variants so
    one [128,510] copy drains 4 blocks
  - DMAs are coalesced: 4x1MB action_count, 1 demand, 1 packed-weight, 11
    zg/rhs0 group streams, 11 output DMAs + ir, all HWDGE on the sync queue
  - embedding lookup zg = W0b[loc] is host-side layout prep (no usable
    device gather), streamed bf16
ir (+b2) is reduced on device (ones column in the zb matmul) and added in
fp32 on the host during unsharding.
"""

import dataclasses as _dc

import numpy as np
import ml_dtypes

import concourse.bass as bass
import concourse.mybir as mybir
from concourse.bass_utils import run_bass_kernel_spmd

BF16 = ml_dtypes.bfloat16
F32 = mybir.dt.float32
BF = mybir.dt.bfloat16

S = 81          # grid states
GRID = 9
N_AG = 20       # agents
A = 5           # actions
B = 32768
NCORES = 8
BC = B // NCORES      # 4096 batch rows per core
NBB = 8               # b-blocks per core (phase N)
BB = BC // NBB        # 512
NG = BC * 5           # 20480 (b, g) columns per core
CB = 510              # col-block: multiple of 5, fits one PSUM bank
NCB = (NG + CB - 1) // CB   # 41 (last block = 80 cols)
NGRP = (NCB + 3) // 4       # 11 groups of <=4 blocks
GW = 4 * CB                 # 2040 cols per group stream
NGP = NGRP * GW             # 22440 padded column count

# wpack column offsets (all lhsT stored in one [128, NWCOL] sbuf tensor)
WIN_OFF = 0            # 4 x 81 inflow chunks
ZBL_OFF = 324          # [81->128, 128] W0a at cols 0..31
IRL_OFF = ZBL_OFF + 128   # ones column at col 32
W0LE_OFF = IRL_OFF + 128  # rows 0..24: w0le
WREP_OFF = W0LE_OFF + 128  # rows 0..31: 4x identity
W1_OFF = WREP_OFF + 128
W2_OFF = W1_OFF + 128      # 4 variants x 128 cols
NWCOL = W2_OFF + 4 * 128   # 1476


def _grid_inflow():
    moves = [(0, 0), (-1, 0), (1, 0), (0, -1), (0, 1)]
    inflow = np.zeros((S * A, S), np.float32)
    for s in range(S):
        r, c = divmod(s, GRID)
        for a, (dr, dc) in enumerate(moves):
            nr, nc_ = r + dr, c + dc
            d = nr * GRID + nc_ if (0 <= nr < GRID and 0 <= nc_ < GRID) else s
            inflow[s * A + a, d] = 1.0
    return inflow


def _cw(k):
    return min(CB, NG - k * CB)


def _gc(t):
    return min(4, NCB - 4 * t)


def _build():
    nc = bass.Bass()

    acT = nc.declare_dram_parameter("acT", [4, 128, 4, 1024], BF, isOutput=False)
    demT = nc.declare_dram_parameter("demT", [S, BC], BF, isOutput=False)
    wpack = nc.declare_dram_parameter("wpack", [128, NWCOL], BF, isOutput=False)
    zgb = nc.declare_dram_parameter("zgb", [128, NGP], BF, isOutput=False)
    rhs0b = nc.declare_dram_parameter("rhs0b", [25, NGP], BF, isOutput=False)
    out4 = nc.declare_dram_parameter("out4", [NGRP, 128, 512], F32, isOutput=True)
    irO = nc.declare_dram_parameter("irO", [1, BC], F32, isOutput=True)

    from contextlib import ExitStack
    ctx = ExitStack()
    with ctx:
        s_ac = ctx.enter_context(nc.sbuf_tensor([128, 4 * BC], BF))
        s_dem = ctx.enter_context(nc.sbuf_tensor([S, BC], BF))
        s_nscT = ctx.enter_context(nc.sbuf_tensor([128, BC], BF))
        s_srv = ctx.enter_context(nc.sbuf_tensor([128, 2 * BB], BF))
        s_zb = ctx.enter_context(nc.sbuf_tensor([128, BC], BF))
        s_ir = ctx.enter_context(nc.sbuf_tensor([1, BC], F32))
        s_zgr = ctx.enter_context(nc.sbuf_tensor([128, 2 * GW], BF))
        s_r0r = ctx.enter_context(nc.sbuf_tensor([128, 2 * GW], BF))
        s_h0 = ctx.enter_context(nc.sbuf_tensor([128, 3 * CB], BF))
        s_h1 = ctx.enter_context(nc.sbuf_tensor([128, 2 * CB], BF))
        s_out = ctx.enter_context(nc.sbuf_tensor([128, NGRP * 512], F32))
        s_wp = ctx.enter_context(nc.sbuf_tensor([128, NWCOL], BF))
        psum = ctx.enter_context(nc.psum_tensor([128, 4096], F32))

        d_w = ctx.enter_context(nc.semaphore())
        d_ac = [ctx.enter_context(nc.semaphore(f"d_ac{i}")) for i in range(4)]
        d_dem = ctx.enter_context(nc.semaphore())
        d_m = [ctx.enter_context(nc.semaphore(f"d_m{i}")) for i in range(2)]
        d_out = ctx.enter_context(nc.semaphore())
        t_nsc = ctx.enter_context(nc.semaphore())
        t_zbir = ctx.enter_context(nc.semaphore())
        t_h0 = ctx.enter_context(nc.semaphore())
        t_h1 = ctx.enter_context(nc.semaphore())
        t_o = ctx.enter_context(nc.semaphore())
        v_ms = ctx.enter_context(nc.semaphore())
        v_nsc = ctx.enter_context(nc.semaphore())
        v_min = ctx.enter_context(nc.semaphore())
        v_h0 = ctx.enter_context(nc.semaphore())
        sc_zbir = ctx.enter_context(nc.semaphore())
        sc_h1 = ctx.enter_context(nc.semaphore())
        sc_o = ctx.enter_context(nc.semaphore())
        block = ctx.enter_context(nc.Block())

        # psum bank map (bank = 512-f32 column chunk)
        def p_h0(k):
            return psum[:, (k % 2) * 512:(k % 2) * 512 + _cw(k)]

        def p_h1(k):
            return psum[:, 1024 + (k % 2) * 512:1024 + (k % 2) * 512 + _cw(k)]

        def p_o(t, cw):
            return psum[:, 2048 + (t % 2) * 512:2048 + (t % 2) * 512 + cw]

        def p_zbir(i):
            return psum[:, 2048 + (i % 2) * 512:2048 + (i % 2) * 512 + BB]

        def p_nsc(i):
            return psum[0:S, 3072 + (i % 2) * 512:3072 + (i % 2) * 512 + BB]

        def wp(off, ncol=128):
            return s_wp[0:128, off:off + ncol]

        def zb_bcast(k):
            cw = _cw(k)
            nb = cw // 5
            b0 = k * (CB // 5)
            ap = s_zb[0:128, b0:b0 + nb]
            return _dc.replace(ap, ap=[ap.ap[0], [1, nb], [0, 5]])

        # zb rows ready for block k: need zb copies of i-blocks covering
        # batch rows < b_end
        def f_zb(k):
            b_end = (k * CB + _cw(k)) // 5
            return (b_end + BB - 1) // BB

        # last block index of group t
        def k_last(t):
            return 4 * t + _gc(t) - 1

        @block.sync
        def _(sync):
            # ac igroup 0 first so phase N can start ASAP
            for ig in range(4):
                dst = s_ac[:, ig * 1024:ig * 1024 + 1024]
                dst = _dc.replace(dst, ap=[dst.ap[0], [4096, 4], [1, 1024]])
                sync.dma_start(dst, acT[ig]).then_inc(d_ac[ig], 16)
                if ig == 0:
                    sync.dma_start(s_wp[:, :], wpack[:, :]).then_inc(d_w, 16)
                    sync.dma_start(s_dem[:, :], demT[:, :]).then_inc(d_dem, 16)
            sync.wait_ge(v_ms, 2)  # padding + s_out memsets done
            for t in range(NGRP + 2):
                if t < NGRP:
                    if t >= 2:
                        sync.wait_ge(v_h0, 2 * k_last(t - 2) + 1)  # slot free
                    sl = slice((t % 2) * GW, (t % 2) * GW + GW)
                    sync.dma_start(s_zgr[:, sl], zgb[:, t * GW:(t + 1) * GW]
                                   ).then_inc(d_m[t % 2], 16)
                    sync.dma_start(s_r0r[0:25, sl], rhs0b[:, t * GW:(t + 1) * GW]
                                   ).then_inc(d_m[t % 2], 16)
                if t == 2:
                    sync.wait_ge(sc_zbir, 2 * NBB)
                    sync.dma_start(irO[:, :], s_ir[:, :]).then_inc(d_out, 16)
                if 2 <= t:
                    to = t - 2
                    sync.wait_ge(sc_o, to + 1)
                    sync.dma_start(out4[to], s_out[:, to * 512:(to + 1) * 512]
                                   ).then_inc(d_out, 16)
            sync.wait_ge(d_out, 16 * (NGRP + 1))

        @block.tensor
        def _(tensor):
            tensor.wait_ge(d_w, 16)
            tensor.wait_ge(v_ms, 1)

            def nsc(i):
                tensor.wait_ge(d_ac[i // 2], 16)
                if i >= 2:
                    tensor.wait_ge(v_nsc, i - 1)
                for c in range(4):
                    mm = nc.tensor.matmul(
                        p_nsc(i), wp(WIN_OFF + c * S, S),
                        s_ac[:, c * BC + i * BB:c * BC + (i + 1) * BB],
                        start=(c == 0), stop=(c == 3),
                    )
                mm.then_inc(t_nsc, 1)

            def zbir(i):
                tensor.wait_ge(v_nsc, i + 1)
                if i >= 2:
                    tensor.wait_ge(sc_zbir, 2 * i - 2)
                nc.tensor.matmul(
                    p_zbir(i), wp(ZBL_OFF), s_nscT[:, i * BB:(i + 1) * BB],
                    start=True, stop=False,
                )
                tensor.wait_ge(v_min, i + 1)
                nc.tensor.matmul(
                    p_zbir(i), wp(IRL_OFF),
                    s_srv[:, (i % 2) * BB:(i % 2 + 1) * BB],
                    start=False, stop=True,
                ).then_inc(t_zbir, 1)

            # phase N, software-skewed
            nsc(0)
            for i in range(1, NBB):
                nsc(i)
                zbir(i - 1)
            zbir(NBB - 1)

            def h0(k):
                t = k // 4
                q = k % 4
                cw = _cw(k)
                tensor.wait_ge(d_m[t % 2], 32 * (t // 2 + 1))
                tensor.wait_ge(sc_zbir, 2 * f_zb(k) - 1)
                if k >= 2:
                    tensor.wait_ge(v_h0, 2 * (k - 2) + 1)  # p_h0 bank free
                nc.tensor.matmul(
                    p_h0(k), wp(W0LE_OFF),
                    s_r0r[0:128, (t % 2) * GW + q * CB:(t % 2) * GW + q * CB + cw],
                    start=True, stop=False,
                )
                nc.tensor.matmul(
                    p_h0(k), wp(WREP_OFF), zb_bcast(k),
                    start=False, stop=True,
                ).then_inc(t_h0, 1)

            def h1(k):
                cw = _cw(k)
                tensor.wait_ge(v_h0, 2 * k + 2)
                if k >= 2:
                    tensor.wait_ge(sc_h1, k - 1)  # p_h1 bank free
                nc.tensor.matmul(
                    p_h1(k), wp(W1_OFF),
                    s_h0[:, (k % 3) * CB:(k % 3) * CB + cw],
                    start=True, stop=True,
                ).then_inc(t_h1, 1)

            def o(k):
                t = k // 4
                q = k % 4
                cw = _cw(k)
                tensor.wait_ge(sc_h1, k + 1)
                if q == 0 and t >= 2:
                    tensor.wait_ge(sc_o, t - 1)  # p_o bank free
                nc.tensor.matmul(
                    p_o(t, cw), wp(W2_OFF + q * 128),
                    s_h1[:, (k % 2) * CB:(k % 2) * CB + cw],
                    start=(q == 0), stop=(q == _gc(t) - 1),
                ).then_inc(t_o, 1)

            # phase M, software-pipelined: h0(kk) | h1(kk-1) | o(kk-2)
            for kk in range(NCB + 2):
                if kk < NCB:
                    h0(kk)
                if 1 <= kk <= NCB:
                    h1(kk - 1)
                if 2 <= kk:
                    o(kk - 2)

        @block.vector
        def _(vector):
            # zero the K-padding rows once; all matmul K dims are then 128
            nc.vector.memset(s_r0r[:, :], 0.0)
            nc.vector.memset(s_nscT[:, :], 0.0)
            nc.vector.memset(s_srv[:, :], 0.0)
            nc.vector.memset(s_zb[:, :], 0.0).then_inc(v_ms, 1)
            nc.vector.memset(s_out[:, :], 0.0).then_inc(v_ms, 1)
            for i in range(NBB):
                vector.wait_ge(t_nsc, i + 1)
                nc.vector.tensor_copy(
                    s_nscT[0:S, i * BB:(i + 1) * BB], p_nsc(i)
                ).then_inc(v_nsc, 1)
                if i == 0:
                    vector.wait_ge(d_dem, 16)
                if i >= 2:
                    vector.wait_ge(t_zbir, i - 1)  # s_srv slot free
                vector.wait_ge(v_nsc, i + 1)  # same-engine RAW
                nc.vector.tensor_tensor(
                    s_srv[0:S, (i % 2) * BB:(i % 2 + 1) * BB],
                    s_nscT[0:S, i * BB:(i + 1) * BB],
                    s_dem[:, i * BB:(i + 1) * BB],
                    mybir.AluOpType.min,
                ).then_inc(v_min, 1)
            for k in range(NCB):
                t = k // 4
                q = k % 4
                cw = _cw(k)
                sl = slice((k % 3) * CB, (k % 3) * CB + cw)
                vector.wait_ge(t_h0, k + 1)
                if k >= 3:
                    vector.wait_ge(t_h1, k - 2)  # s_h0 slot free
                nc.vector.tensor_tensor(
                    s_h0[:, sl], p_h0(k),
                    s_zgr[:, (t % 2) * GW + q * CB:(t % 2) * GW + q * CB + cw],
                    mybir.AluOpType.add,
                ).then_inc(v_h0, 1)
                vector.wait_ge(v_h0, 2 * k + 1)  # same-engine RAW
                nc.vector.tensor_scalar_max(
                    s_h0[:, sl], s_h0[:, sl], 0.0
                ).then_inc(v_h0, 1)

        @block.scalar
        def _(scalar):
            AF = mybir.ActivationFunctionType
            scalar.wait_ge(v_ms, 2)  # s_out memset before group copies
            for i in range(NBB):
                scalar.wait_ge(t_zbir, i + 1)
                pz = p_zbir(i)
                nc.scalar.copy(
                    s_zb[0:32, i * BB:(i + 1) * BB], pz[0:32, :]
                ).then_inc(sc_zbir, 1)
                nc.scalar.copy(
                    s_ir[:, i * BB:(i + 1) * BB], pz[32:33, :]
                ).then_inc(sc_zbir, 1)
            for k in range(NCB):
                t = k // 4
                q = k % 4
                cw = _cw(k)
                scalar.wait_ge(t_h1, k + 1)
                if k >= 2:
                    scalar.wait_ge(t_o, k - 1)  # s_h1 slot free
                nc.scalar.activation(
                    s_h1[:, (k % 2) * CB:(k % 2) * CB + cw], p_h1(k), AF.Relu
                ).then_inc(sc_h1, 1)
                if q == _gc(t) - 1:
                    scalar.wait_ge(t_o, k + 1)
                    cwg = _cw(4 * t)  # width of the group's blocks
                    nc.scalar.copy(
                        s_out[:, t * 512:t * 512 + cwg], p_o(t, cwg)
                    ).then_inc(sc_o, 1)

    return nc


_NC = {}


def _get_nc():
    if "v2" not in _NC:
        _NC["v2"] = _build()
    return _NC["v2"]


def _prep_core(obs, ac, la, zg_tab):
    """Host-side layout prep for one core's batch slice (all numpy)."""
    bc = obs.shape[0]
    out = {}
    # acT: [405, bc] padded to [512, bc] -> [ig, p, c, 1024]
    acf = np.zeros((512, bc), np.float32)
    acf[:405] = ac.reshape(bc, 405).T
    out["acT"] = np.ascontiguousarray(
        acf.reshape(4, 128, 4, 1024).transpose(2, 1, 0, 3)
    ).astype(BF16)
    # demT: [81, bc]
    out["demT"] = np.ascontiguousarray(obs[:, S:2 * S].T).astype(BF16)
    # zg: [128, NG] stacked by agent class, padded to NGP
    loc = obs[:, 2 * S:2 * S + N_AG].astype(np.int64)  # [bc, 20]
    zst = np.zeros((128, NGP), np.float32)
    for j in range(4):
        lj = loc[:, j::4].reshape(-1)            # cols (b, g)
        zst[32 * j:32 * j + 32, :bc * 5] = zg_tab[lj].T
    out["zgb"] = zst.astype(BF16)
    # rhs0: rows 0-19 la packed, rows 20-24 g-onehot; [25, NGP]
    r0 = np.zeros((25, NGP), np.float32)
    for j in range(4):
        r0[5 * j:5 * j + 5, :bc * 5] = (
            la[:, j::4, :].transpose(2, 0, 1).reshape(5, bc * 5)
        )
    r0[20:25, :bc * 5] = np.tile(np.eye(5, dtype=np.float32), (1, bc))
    out["rhs0b"] = r0.astype(BF16)
    return out


def kernel(obs, action_count, local_actions, W0, W1, W2, b2):
    obs = np.asarray(obs, np.float32)
    action_count = np.asarray(action_count, np.float32)
    local_actions = np.asarray(local_actions, np.float32)
    W0 = np.asarray(W0, np.float32)
    W1 = np.asarray(W1, np.float32)
    W2 = np.asarray(W2, np.float32)
    b2 = np.asarray(b2, np.float32)

    # ---- weight repacking (shared across cores) ----
    W0a, W0b = W0[0:S], W0[S:2 * S]           # [81,32], [81,32]
    W0c, W0d = W0[2 * S:2 * S + A], W0[2 * S + A:]  # [5,32], [20,32]
    inflow = _grid_inflow()
    wpk = np.zeros((128, NWCOL), np.float32)
    infp = np.zeros((512, S), np.float32)
    infp[:405] = inflow
    for c in range(4):
        wpk[:, WIN_OFF + c * S:WIN_OFF + (c + 1) * S] = infp[c * 128:(c + 1) * 128]
    wpk[0:S, ZBL_OFF:ZBL_OFF + 32] = W0a
    wpk[0:S, IRL_OFF + 32] = 1.0
    for j in range(4):
        wpk[5 * j:5 * j + 5, W0LE_OFF + 32 * j:W0LE_OFF + 32 * j + 32] = W0c
        for e in range(5):
            wpk[20 + e, W0LE_OFF + 32 * j:W0LE_OFF + 32 * j + 32] = W0d[4 * e + j]
        wpk[0:32, WREP_OFF + 32 * j:WREP_OFF + 32 * j + 32] = np.eye(32)
        wpk[32 * j:32 * j + 32, W1_OFF + 32 * j:W1_OFF + 32 * j + 32] = W1
    for q in range(4):
        for j in range(4):
            wpk[32 * j:32 * j + 32, W2_OFF + 128 * q + 32 * q + j] = W2[:, 0]
    wpack = wpk.astype(BF16)

    in_maps = []
    for c in range(NCORES):
        bsl = slice(c * BC, (c + 1) * BC)
        m = _prep_core(obs[bsl], action_count[bsl], local_actions[bsl], W0b)
        m["wpack"] = wpack
        in_maps.append(m)

    nc = _get_nc()
    res = run_bass_kernel_spmd(nc, in_maps, list(range(NCORES)))
    global LAST_RESULTS
    LAST_RESULTS = res

    out = np.empty((B, N_AG), np.float32)
    for c in range(NCORES):
        r = res.results[c]
        o = r["out4"][:, :, :CB].reshape(NGRP, 4, 32, CB)[:, :, 0:4, :]
        o = o.transpose(0, 1, 3, 2).reshape(NGRP * 4 * CB, 4)[:NG]
        ob = o.reshape(BC, 5, 4).reshape(BC, N_AG)  # n = 4g + j
        ob = ob + r["irO"][0][:, None] + b2[0]
        out[c * BC:(c + 1) * BC] = ob
    return out


# revision 14
# speedup vs baseline: 2.0738x; 1.1250x over previous
"""Trainium2 Bass kernel for DecGridDeepVPN (gnn_message_passing), 8-core SPMD.

Math (per batch row b, agents n=0..19):
  nsc[b]  = action_count[b].reshape(405) @ INFLOW            # [81]
  ir[b]   = sum_d min(nsc[b,d], demand[b,d])                 # scalar
  feat    = [nsc | onehot81(loc[b,n]) | la[b,n] | onehot20(n)]   # 187
  h0 = relu(feat @ W0); h1 = relu(h0 @ W1); out[b,n] = h1@W2 + b2 + ir[b]

Sharding: pure data-parallel over batch (4096 rows/core). Feature-major on
device: features on partitions, batch streaming on the free axis. MLP columns
are (b, g) pairs with g = n//4; agent classes j = n%4 occupy four 32-row
partition blocks, so all layers are K=128 block-diagonal matmuls.

v2 design (from trace analysis of v1: PE 80% cold-throttled, scalar engine
at 98us of PSUM-copy work, 174 small DMAs):
  - every matmul is K-padded to 128 with zeroed SBUF rows / weight rows so
    the PE never switches tile mode (no drain, stays HAM-warm)
  - the h0 = relu(presum + zg) add reads PSUM directly on the vector engine
    (no staging copy); relu-h1 on scalar; outputs of 4 consecutive column
    blocks are accumulated into one PSUM bank via sparse-M W2 variants so
    one [128,510] copy drains 4 blocks
  - DMAs are coalesced: 4x1MB action_count, 1 demand, 1 packed-weight, 11
    zg/rhs0 group streams, 11 output DMAs + ir, all HWDGE on the sync queue
  - embedding lookup zg = W0b[loc] is host-side layout prep (no usable
    device gather), streamed bf16
ir (+b2) is reduced on device (ones column in the zb matmul) and added in
fp32 on the host during unsharding.
"""

import dataclasses as _dc

import numpy as np
import ml_dtypes

import concourse.bass as bass
import concourse.mybir as mybir
from concourse.bass_utils import run_bass_kernel_spmd

BF16 = ml_dtypes.bfloat16
F32 = mybir.dt.float32
BF = mybir.dt.bfloat16

S = 81          # grid states
GRID = 9
N_AG = 20       # agents
A = 5           # actions
B = 32768
NCORES = 8
BC = B // NCORES      # 4096 batch rows per core
NBB = 8               # b-blocks per core (phase N)
BB = BC // NBB        # 512
NG = BC * 5           # 20480 (b, g) columns per core
CB = 510              # col-block: multiple of 5, fits one PSUM bank
NCB = (NG + CB - 1) // CB   # 41 (last block = 80 cols)
NGRP = (NCB + 3) // 4       # 11 groups of <=4 blocks
GW = 4 * CB                 # 2040 cols per group stream
NGP = NGRP * GW             # 22440 padded column count

# wpack column offsets (all lhsT stored in one [128, NWCOL] sbuf tensor)
WIN_OFF = 0            # 4 x 81 inflow chunks
ZBL_OFF = 324          # [81->128, 128] W0a at cols 0..31
IRL_OFF = ZBL_OFF + 128   # ones column at col 32
W0LE_OFF = IRL_OFF + 128  # rows 0..24: w0le
WREP_OFF = W0LE_OFF + 128  # rows 0..31: 4x identity
W1_OFF = WREP_OFF + 128
W2_OFF = W1_OFF + 128      # 4 variants x 128 cols
NWCOL = W2_OFF + 4 * 128   # 1476


def _grid_inflow():
    moves = [(0, 0), (-1, 0), (1, 0), (0, -1), (0, 1)]
    inflow = np.zeros((S * A, S), np.float32)
    for s in range(S):
        r, c = divmod(s, GRID)
        for a, (dr, dc) in enumerate(moves):
            nr, nc_ = r + dr, c + dc
            d = nr * GRID + nc_ if (0 <= nr < GRID and 0 <= nc_ < GRID) else s
            inflow[s * A + a, d] = 1.0
    return inflow


def _cw(k):
    return min(CB, NG - k * CB)


def _gc(t):
    return min(4, NCB - 4 * t)


def _build():
    nc = bass.Bass()

    acT = nc.declare_dram_parameter("acT", [4, 128, 4, 1024], BF, isOutput=False)
    demT = nc.declare_dram_parameter("demT", [S, BC], BF, isOutput=False)
    wpack = nc.declare_dram_parameter("wpack", [128, NWCOL], BF, isOutput=False)
    zgb = nc.declare_dram_parameter("zgb", [128, NGP], BF, isOutput=False)
    rhs0b = nc.declare_dram_parameter("rhs0b", [25, NGP], BF, isOutput=False)
    out4 = nc.declare_dram_parameter("out4", [NGRP, 128, 512], F32, isOutput=True)
    irO = nc.declare_dram_parameter("irO", [1, BC], F32, isOutput=True)

    from contextlib import ExitStack
    ctx = ExitStack()
    with ctx:
        s_ac = ctx.enter_context(nc.sbuf_tensor([128, 4 * BC], BF))
        s_dem = ctx.enter_context(nc.sbuf_tensor([S, BC], BF))
        s_nscT = ctx.enter_context(nc.sbuf_tensor([128, BC], BF))
        s_srv = ctx.enter_context(nc.sbuf_tensor([128, 2 * BB], BF))
        s_zb = ctx.enter_context(nc.sbuf_tensor([128, BC], BF))
        s_ir = ctx.enter_context(nc.sbuf_tensor([1, BC], F32))
        s_zgr = ctx.enter_context(nc.sbuf_tensor([128, 2 * GW], BF))
        s_r0r = ctx.enter_context(nc.sbuf_tensor([128, 2 * GW], BF))
        s_h0 = ctx.enter_context(nc.sbuf_tensor([128, 4 * CB], BF))
        s_h1 = ctx.enter_context(nc.sbuf_tensor([128, 4 * CB], BF))
        s_out = ctx.enter_context(nc.sbuf_tensor([128, NGRP * 512], F32))
        s_wp = ctx.enter_context(nc.sbuf_tensor([128, NWCOL], BF))
        psum = ctx.enter_context(nc.psum_tensor([128, 4096], F32))

        d_w = ctx.enter_context(nc.semaphore())
        d_ac = [ctx.enter_context(nc.semaphore(f"d_ac{i}")) for i in range(4)]
        d_dem = ctx.enter_context(nc.semaphore())
        d_m = [ctx.enter_context(nc.semaphore(f"d_m{i}")) for i in range(2)]
        d_out = ctx.enter_context(nc.semaphore())
        t_nsc = ctx.enter_context(nc.semaphore())
        t_zbir = ctx.enter_context(nc.semaphore())
        t_h0 = ctx.enter_context(nc.semaphore())
        t_h1 = ctx.enter_context(nc.semaphore())
        t_o = ctx.enter_context(nc.semaphore())
        g_ms = ctx.enter_context(nc.semaphore())
        v_nsc = ctx.enter_context(nc.semaphore())
        v_min = ctx.enter_context(nc.semaphore())
        v_h0 = ctx.enter_context(nc.semaphore())
        sc_zbir = ctx.enter_context(nc.semaphore())
        sc_h1 = ctx.enter_context(nc.semaphore())
        sc_o = ctx.enter_context(nc.semaphore())
        block = ctx.enter_context(nc.Block())

        # psum bank map (bank = 512-f32 column chunk)
        # phase M: h0 banks 0-3, h1 banks 4-5, o banks 6-7
        # phase N: zbir banks 4-5, nsc banks 6-7 (FIFO/sem ordered vs M)
        def p_h0(k):
            return psum[:, (k % 4) * 512:(k % 4) * 512 + _cw(k)]

        def p_h1(k):
            return psum[:, 2048 + (k % 2) * 512:2048 + (k % 2) * 512 + _cw(k)]

        def p_o(t, cw):
            return psum[:, 3072 + (t % 2) * 512:3072 + (t % 2) * 512 + cw]

        def p_zbir(i):
            return psum[:, 2048 + (i % 2) * 512:2048 + (i % 2) * 512 + BB]

        def p_nsc(i):
            return psum[0:S, 3072 + (i % 2) * 512:3072 + (i % 2) * 512 + BB]

        def wp(off, ncol=128):
            return s_wp[0:128, off:off + ncol]

        def wp81(off, ncol=128):
            return s_wp[0:S, off:off + ncol]

        def zb_bcast(k):
            cw = _cw(k)
            nb = cw // 5
            b0 = k * (CB // 5)
            ap = s_zb[0:128, b0:b0 + nb]
            return _dc.replace(ap, ap=[ap.ap[0], [1, nb], [0, 5]])

        # zb rows ready for block k: need zb copies of i-blocks covering
        # batch rows < b_end
        def f_zb(k):
            b_end = (k * CB + _cw(k)) // 5
            return (b_end + BB - 1) // BB

        # last block index of group t
        def k_last(t):
            return 4 * t + _gc(t) - 1

        @block.sync
        def _(sync):
            # ac igroup 0 first so phase N can start ASAP
            for ig in range(4):
                dst = s_ac[:, ig * 1024:ig * 1024 + 1024]
                dst = _dc.replace(dst, ap=[dst.ap[0], [4096, 4], [1, 1024]])
                sync.dma_start(dst, acT[ig]).then_inc(d_ac[ig], 16)
                if ig == 0:
                    sync.dma_start(s_wp[:, :], wpack[:, :]).then_inc(d_w, 16)
                    sync.dma_start(s_dem[:, :], demT[:, :]).then_inc(d_dem, 16)
            sync.wait_ge(g_ms, 1)  # s_r0r padding memset before r0 DMA
            for t in range(NGRP + 2):
                if t < NGRP:
                    if t >= 2:
                        sync.wait_ge(v_h0, 2 * k_last(t - 2) + 1)  # slot free
                    sl = slice((t % 2) * GW, (t % 2) * GW + GW)
                    sync.dma_start(s_zgr[:, sl], zgb[:, t * GW:(t + 1) * GW]
                                   ).then_inc(d_m[t % 2], 16)
                    sync.dma_start(s_r0r[0:25, sl], rhs0b[:, t * GW:(t + 1) * GW]
                                   ).then_inc(d_m[t % 2], 16)
                if t == 2:
                    sync.wait_ge(sc_zbir, 2 * NBB)
                    sync.dma_start(irO[:, :], s_ir[:, :]).then_inc(d_out, 16)
                if 2 <= t:
                    to = t - 2
                    sync.wait_ge(sc_o, to + 1)
                    sync.dma_start(out4[to], s_out[:, to * 512:(to + 1) * 512]
                                   ).then_inc(d_out, 16)
            sync.wait_ge(d_out, 16 * (NGRP + 1))

        @block.tensor
        def _(tensor):
            tensor.wait_ge(d_w, 16)

            def nsc(i):
                tensor.wait_ge(d_ac[i // 2], 16)
                if i >= 2:
                    tensor.wait_ge(v_nsc, i - 1)
                for c in range(4):
                    mm = nc.tensor.matmul(
                        p_nsc(i), wp(WIN_OFF + c * S, S),
                        s_ac[:, c * BC + i * BB:c * BC + (i + 1) * BB],
                        start=(c == 0), stop=(c == 3),
                    )
                mm.then_inc(t_nsc, 1)

            def zbir(i):
                tensor.wait_ge(v_nsc, i + 1)
                if i >= 2:
                    tensor.wait_ge(sc_zbir, 2 * i - 2)
                nc.tensor.matmul(
                    p_zbir(i), wp81(ZBL_OFF), s_nscT[0:S, i * BB:(i + 1) * BB],
                    start=True, stop=False,
                )
                tensor.wait_ge(v_min, i + 1)
                nc.tensor.matmul(
                    p_zbir(i), wp81(IRL_OFF),
                    s_srv[0:S, (i % 2) * BB:(i % 2 + 1) * BB],
                    start=False, stop=True,
                ).then_inc(t_zbir, 1)

            # phase N, software-skewed
            nsc(0)
            for i in range(1, NBB):
                nsc(i)
                zbir(i - 1)
            zbir(NBB - 1)

            def h0(k):
                t = k // 4
                q = k % 4
                cw = _cw(k)
                if k == 0:
                    tensor.wait_ge(g_ms, 2)  # r0/zb padding memsets
                tensor.wait_ge(d_m[t % 2], 32 * (t // 2 + 1))
                tensor.wait_ge(sc_zbir, 2 * f_zb(k) - 1)
                if k >= 4:
                    tensor.wait_ge(v_h0, 2 * (k - 4) + 1)  # p_h0 bank free
                nc.tensor.matmul(
                    p_h0(k), wp(W0LE_OFF),
                    s_r0r[0:128, (t % 2) * GW + q * CB:(t % 2) * GW + q * CB + cw],
                    start=True, stop=False,
                )
                nc.tensor.matmul(
                    p_h0(k), wp(WREP_OFF), zb_bcast(k),
                    start=False, stop=True,
                ).then_inc(t_h0, 1)

            def h1(k):
                cw = _cw(k)
                tensor.wait_ge(v_h0, 2 * k + 2)
                if k < 2:
                    tensor.wait_ge(sc_zbir, 2 * NBB)  # zbir copies off banks 4-5
                else:
                    tensor.wait_ge(sc_h1, k - 1)  # p_h1 bank free
                nc.tensor.matmul(
                    p_h1(k), wp(W1_OFF),
                    s_h0[:, (k % 4) * CB:(k % 4) * CB + cw],
                    start=True, stop=True,
                ).then_inc(t_h1, 1)

            def o(k):
                t = k // 4
                q = k % 4
                cw = _cw(k)
                tensor.wait_ge(sc_h1, k + 1)
                if q == 0 and t >= 2:
                    tensor.wait_ge(sc_o, t - 1)  # p_o bank free
                nc.tensor.matmul(
                    p_o(t, cw), wp(W2_OFF + q * 128),
                    s_h1[:, (k % 4) * CB:(k % 4) * CB + cw],
                    start=(q == 0), stop=(q == _gc(t) - 1),
                ).then_inc(t_o, 1)

            # phase M, software-pipelined: h0(kk) | h1(kk-2) | o(kk-4)
            for kk in range(NCB + 4):
                if kk < NCB:
                    h0(kk)
                if 2 <= kk < NCB + 2:
                    h1(kk - 2)
                if 4 <= kk:
                    o(kk - 4)

        @block.gpsimd
        def _(gpsimd):
            nc.gpsimd.memset(s_r0r[:, :], 0.0).then_inc(g_ms, 1)
            nc.gpsimd.memset(s_zb[:, :], 0.0).then_inc(g_ms, 1)
            nc.gpsimd.memset(s_out[:, :], 0.0).then_inc(g_ms, 1)

        @block.vector
        def _(vector):
            for i in range(NBB):
                vector.wait_ge(t_nsc, i + 1)
                nc.vector.tensor_copy(
                    s_nscT[0:S, i * BB:(i + 1) * BB], p_nsc(i)
                ).then_inc(v_nsc, 1)
                if i == 0:
                    vector.wait_ge(d_dem, 16)
                if i >= 2:
                    vector.wait_ge(t_zbir, i - 1)  # s_srv slot free
                vector.wait_ge(v_nsc, i + 1)  # same-engine RAW
                nc.vector.tensor_tensor(
                    s_srv[0:S, (i % 2) * BB:(i % 2 + 1) * BB],
                    s_nscT[0:S, i * BB:(i + 1) * BB],
                    s_dem[:, i * BB:(i + 1) * BB],
                    mybir.AluOpType.min,
                ).then_inc(v_min, 1)
            for k in range(NCB):
                t = k // 4
                q = k % 4
                cw = _cw(k)
                sl = slice((k % 4) * CB, (k % 4) * CB + cw)
                vector.wait_ge(t_h0, k + 1)
                if k >= 4:
                    vector.wait_ge(t_h1, k - 3)  # s_h0 slot free
                nc.vector.tensor_tensor(
                    s_h0[:, sl], p_h0(k),
                    s_zgr[:, (t % 2) * GW + q * CB:(t % 2) * GW + q * CB + cw],
                    mybir.AluOpType.add,
                ).then_inc(v_h0, 1)
                vector.wait_ge(v_h0, 2 * k + 1)  # same-engine RAW
                nc.vector.tensor_scalar_max(
                    s_h0[:, sl], s_h0[:, sl], 0.0
                ).then_inc(v_h0, 1)

        @block.scalar
        def _(scalar):
            AF = mybir.ActivationFunctionType
            scalar.wait_ge(g_ms, 3)  # s_out memset before group copies
            for i in range(NBB):
                scalar.wait_ge(t_zbir, i + 1)
                pz = p_zbir(i)
                nc.scalar.copy(
                    s_zb[0:32, i * BB:(i + 1) * BB], pz[0:32, :]
                ).then_inc(sc_zbir, 1)
                nc.scalar.copy(
                    s_ir[:, i * BB:(i + 1) * BB], pz[32:33, :]
                ).then_inc(sc_zbir, 1)
            for k in range(NCB):
                t = k // 4
                q = k % 4
                cw = _cw(k)
                scalar.wait_ge(t_h1, k + 1)
                if k >= 4:
                    scalar.wait_ge(t_o, k - 3)  # s_h1 slot free
                nc.scalar.activation(
                    s_h1[:, (k % 4) * CB:(k % 4) * CB + cw], p_h1(k), AF.Relu
                ).then_inc(sc_h1, 1)
                if q == _gc(t) - 1:
                    scalar.wait_ge(t_o, k + 1)
                    cwg = _cw(4 * t)  # width of the group's blocks
                    nc.scalar.copy(
                        s_out[:, t * 512:t * 512 + cwg], p_o(t, cwg)
                    ).then_inc(sc_o, 1)

    return nc


_NC = {}


def _get_nc():
    if "v2" not in _NC:
        _NC["v2"] = _build()
    return _NC["v2"]


def _prep_core(obs, ac, la, zg_tab):
    """Host-side layout prep for one core's batch slice (all numpy)."""
    bc = obs.shape[0]
    out = {}
    # acT: [405, bc] padded to [512, bc] -> [ig, p, c, 1024]
    acf = np.zeros((512, bc), np.float32)
    acf[:405] = ac.reshape(bc, 405).T
    out["acT"] = np.ascontiguousarray(
        acf.reshape(4, 128, 4, 1024).transpose(2, 1, 0, 3)
    ).astype(BF16)
    # demT: [81, bc]
    out["demT"] = np.ascontiguousarray(obs[:, S:2 * S].T).astype(BF16)
    # zg: [128, NG] stacked by agent class, padded to NGP
    loc = obs[:, 2 * S:2 * S + N_AG].astype(np.int64)  # [bc, 20]
    zst = np.zeros((128, NGP), np.float32)
    for j in range(4):
        lj = loc[:, j::4].reshape(-1)            # cols (b, g)
        zst[32 * j:32 * j + 32, :bc * 5] = zg_tab[lj].T
    out["zgb"] = zst.astype(BF16)
    # rhs0: rows 0-19 la packed, rows 20-24 g-onehot; [25, NGP]
    r0 = np.zeros((25, NGP), np.float32)
    for j in range(4):
        r0[5 * j:5 * j + 5, :bc * 5] = (
            la[:, j::4, :].transpose(2, 0, 1).reshape(5, bc * 5)
        )
    r0[20:25, :bc * 5] = np.tile(np.eye(5, dtype=np.float32), (1, bc))
    out["rhs0b"] = r0.astype(BF16)
    return out


def kernel(obs, action_count, local_actions, W0, W1, W2, b2):
    obs = np.asarray(obs, np.float32)
    action_count = np.asarray(action_count, np.float32)
    local_actions = np.asarray(local_actions, np.float32)
    W0 = np.asarray(W0, np.float32)
    W1 = np.asarray(W1, np.float32)
    W2 = np.asarray(W2, np.float32)
    b2 = np.asarray(b2, np.float32)

    # ---- weight repacking (shared across cores) ----
    W0a, W0b = W0[0:S], W0[S:2 * S]           # [81,32], [81,32]
    W0c, W0d = W0[2 * S:2 * S + A], W0[2 * S + A:]  # [5,32], [20,32]
    inflow = _grid_inflow()
    wpk = np.zeros((128, NWCOL), np.float32)
    infp = np.zeros((512, S), np.float32)
    infp[:405] = inflow
    for c in range(4):
        wpk[:, WIN_OFF + c * S:WIN_OFF + (c + 1) * S] = infp[c * 128:(c + 1) * 128]
    wpk[0:S, ZBL_OFF:ZBL_OFF + 32] = W0a
    wpk[0:S, IRL_OFF + 32] = 1.0
    for j in range(4):
        wpk[5 * j:5 * j + 5, W0LE_OFF + 32 * j:W0LE_OFF + 32 * j + 32] = W0c
        for e in range(5):
            wpk[20 + e, W0LE_OFF + 32 * j:W0LE_OFF + 32 * j + 32] = W0d[4 * e + j]
        wpk[0:32, WREP_OFF + 32 * j:WREP_OFF + 32 * j + 32] = np.eye(32)
        wpk[32 * j:32 * j + 32, W1_OFF + 32 * j:W1_OFF + 32 * j + 32] = W1
    for q in range(4):
        for j in range(4):
            wpk[32 * j:32 * j + 32, W2_OFF + 128 * q + 32 * q + j] = W2[:, 0]
    wpack = wpk.astype(BF16)

    in_maps = []
    for c in range(NCORES):
        bsl = slice(c * BC, (c + 1) * BC)
        m = _prep_core(obs[bsl], action_count[bsl], local_actions[bsl], W0b)
        m["wpack"] = wpack
        in_maps.append(m)

    nc = _get_nc()
    res = run_bass_kernel_spmd(nc, in_maps, list(range(NCORES)))
    global LAST_RESULTS
    LAST_RESULTS = res

    out = np.empty((B, N_AG), np.float32)
    for c in range(NCORES):
        r = res.results[c]
        o = r["out4"][:, :, :CB].reshape(NGRP, 4, 32, CB)[:, :, 0:4, :]
        o = o.transpose(0, 1, 3, 2).reshape(NGRP * 4 * CB, 4)[:NG]
        ob = o.reshape(BC, 5, 4).reshape(BC, N_AG)  # n = 4g + j
        ob = ob + r["irO"][0][:, None] + b2[0]
        out[c * BC:(c + 1) * BC] = ob
    return out
